# revision 1
# baseline (speedup 1.0000x reference)
"""DSFourierAttention Trainium2 kernel.

Math (per (b, h) slice, validated vs the jax reference):
    qf = rfft(q, ortho) etc. as dense DFT matmuls (Fre/Fim [L, X], X = L//2+1)
    qk_T[y, x] = sum_e (kfr qfr + kfi qfi)       (stacked [re; im] K=128 matmul)
    im_T[y, x] = sum_e (-kfi qfr + kfr qfi)      (kf_swap = [-kfi; kfr])
    p = exp(sqrt(re^2 + im^2))                   (no max subtraction; |qk| <= ~5)
    qkv_T[x, e] = (p^T @ [vfr | vfi | ones]) / colsum   (ones col gives colsum)
    out[l, e] = Gre^T @ qkvr + Gim^T @ qkvi      (irfft weights w = [1, 2.., 2, 1])
    out = out * tau[b] + delta[b, l]

Sharding: batch-parallel, 2 batches per core across 8 cores.
All big matmuls run as float32r (1 cyc/row at N>=256); AV + iFFT run bf16.
"""

import os
import sys

import numpy as np

for _p in ("/opt/trn_rl_repo", "/root/.axon_site/_ro/trn_rl_repo"):
    if os.path.isdir(_p) and _p not in sys.path:
        sys.path.insert(0, _p)

import ml_dtypes  # noqa: E402
import concourse.bass as bass  # noqa: E402
import concourse.tile as tile  # noqa: E402
from concourse import bacc, mybir  # noqa: E402
from concourse.bass_utils import run_bass_kernel_spmd  # noqa: E402

B, L, H, E = 16, 1024, 8, 64
X = L // 2 + 1          # 513 rfft bins
XP = X + 1              # padded to 514: fp32r matmul dst free size must be >= 2
NCORES = 8
BL = B // NCORES        # 2 batches per core
NLC = L // 128          # 8 l-chunks
NYC = 4                 # full 128-row y chunks (y=512 handled as ragged row)
NXC = 4                 # full 128-row x chunks (x=512 handled as ragged row)
NWAVE = 2               # ACT-table waves per batch (4 slices each)

F32 = mybir.dt.float32
BF16 = mybir.dt.bfloat16
F32R = mybir.dt.float32r
AF = mybir.ActivationFunctionType
OP = None  # set after import in build

LAST_RESULT = None


def _r(ap):
    """View an fp32 AP as float32r for fast PE matmuls (unused: tiles are
    allocated as float32r directly so the BIR verifier sees rounded
    producers)."""
    return ap.bitcast(F32R)


def _consts():
    l = np.arange(L)
    xs = np.arange(X)
    ang = 2.0 * np.pi * np.outer(l, xs) / L          # [L, X]
    fre = np.zeros((L, XP), ml_dtypes.bfloat16)
    fim = np.zeros((L, XP), ml_dtypes.bfloat16)
    fre[:, :X] = (np.cos(ang) / np.sqrt(L)).astype(ml_dtypes.bfloat16)
    fim[:, :X] = (-np.sin(ang) / np.sqrt(L)).astype(ml_dtypes.bfloat16)
    w = np.full(X, 2.0)
    w[0] = 1.0
    w[-1] = 1.0
    gre = (w[:, None] * np.cos(ang.T) / np.sqrt(L)).astype(ml_dtypes.bfloat16)
    gim = (w[:, None] * -np.sin(ang.T) / np.sqrt(L)).astype(ml_dtypes.bfloat16)
    return fre, fim, gre, gim


def build_module(bl=BL, compile=True):
    from concourse.alu_op_type import AluOpType

    nc = bacc.Bacc("TRN2", target_bir_lowering=False, debug=False,
                   num_devices=NCORES)

    qd = nc.dram_tensor("qd", [bl, L, H, E], BF16, kind="ExternalInput").ap()
    kd = nc.dram_tensor("kd", [bl, L, H, E], BF16, kind="ExternalInput").ap()
    vd = nc.dram_tensor("vd", [bl, L, H, E], BF16, kind="ExternalInput").ap()
    taud = nc.dram_tensor("taud", [bl, 1], F32, kind="ExternalInput").ap()
    deltad = nc.dram_tensor("deltad", [bl, L], F32, kind="ExternalInput").ap()
    fred = nc.dram_tensor("fred", [L, XP], BF16, kind="ExternalInput").ap()
    fimd = nc.dram_tensor("fimd", [L, XP], BF16, kind="ExternalInput").ap()
    gred = nc.dram_tensor("gred", [X, L], BF16, kind="ExternalInput").ap()
    gimd = nc.dram_tensor("gimd", [X, L], BF16, kind="ExternalInput").ap()
    outd = nc.dram_tensor("outd", [bl, L, H, E], F32, kind="ExternalOutput").ap()

    with tile.TileContext(nc) as tc:
        _body(nc, tc, AluOpType, qd, kd, vd, taud, deltad, fred, fimd, gred,
              gimd, outd, bl)
    if compile:
        nc.compile()
    return nc


def _body(nc, tc, OPS, qd, kd, vd, taud, deltad, fred, fimd, gred, gimd, outd,
          bl=BL):
    from contextlib import ExitStack

    ctx = ExitStack()
    with ctx:
        consts = ctx.enter_context(tc.tile_pool(name="consts", bufs=1))
        io = ctx.enter_context(tc.tile_pool(name="io", bufs=2))
        stg = ctx.enter_context(tc.tile_pool(name="stg", bufs=2))
        stk = ctx.enter_context(tc.tile_pool(name="stk", bufs=4))
        wv = ctx.enter_context(tc.tile_pool(name="wv", bufs=17))
        sm = ctx.enter_context(tc.tile_pool(name="sm", bufs=3))
        vfp = ctx.enter_context(tc.tile_pool(name="vfp", bufs=5))
        qkvp = ctx.enter_context(tc.tile_pool(name="qkvp", bufs=5))
        ep = ctx.enter_context(tc.tile_pool(name="ep", bufs=2))
        pf = ctx.enter_context(tc.tile_pool(name="pf", bufs=3, space="PSUM"))
        ph = ctx.enter_context(tc.tile_pool(name="ph", bufs=2, space="PSUM"))

        # ---- constants -------------------------------------------------
        fre_sb = consts.tile([128, NLC, XP], BF16)
        fim_sb = consts.tile([128, NLC, XP], BF16)
        for c in range(NLC):
            nc.sync.dma_start(
                out=fre_sb[:, c, :],
                in_=fred.rearrange("(c p) x -> p c x", p=128)[:, c, :])
            nc.sync.dma_start(
                out=fim_sb[:, c, :],
                in_=fimd.rearrange("(c p) x -> p c x", p=128)[:, c, :])
        gre512 = consts.tile([1, L], BF16)
        nc.sync.dma_start(out=gre512[0:1, :], in_=gred[512:513, :])
        gst = ctx.enter_context(tc.tile_pool(name="gst", bufs=4))

        vf_next = _phase_v(nc, 0, vd, fre_sb, fim_sb, io, vfp, pf, ph)
        for b in range(bl):
            vf_next = _batch(nc, tc, OPS, b, bl, qd, kd, vd, taud, deltad,
                             outd, fre_sb, fim_sb, gred, gimd, gre512, gst,
                             io, stg, stk, wv, sm, vfp, qkvp, ep, pf, ph,
                             vf_next)


def _phase_v(nc, b, vd, fre_sb, fim_sb, io, vfp, pf, ph):
    """Load v[b] and compute the transposed FFT into vf_av/v512 tiles."""
    v_sb = io.tile([128, NLC, H * E], BF16, tag="vsb", bufs=2,
                   name=f"vsb{b}")
    for c in range(NLC):
        nc.sync.dma_start(
            out=v_sb[:, c, :],
            in_=vd[b].rearrange("(c p) h e -> p c (h e)", p=128)[:, c, :])

    vf_av = []
    for yc in range(NYC):
        t = vfp.tile([128, H, 132], BF16, tag="vfav", bufs=8,
                     name=f"vfav{b}_{yc}")
        vf_av.append(t)
    v512 = vfp.tile([1, H, 132], BF16, tag="v512", bufs=2, name=f"v512_{b}")

    for part, f_sb in ((0, fre_sb), (1, fim_sb)):
        for yc in range(NYC):
            ps = pf.tile([128, 520], F32, tag="pf", name=f"psv{b}_{part}_{yc}")
            for c in range(NLC):
                nc.tensor.matmul(ps[:, 0:512],
                                 f_sb[:, c, yc * 128:(yc + 1) * 128],
                                 v_sb[:, c, :],
                                 start=(c == 0), stop=(c == NLC - 1))
            nc.vector.tensor_copy(
                out=vf_av[yc][:, :, part * 64:(part + 1) * 64],
                in_=ps[:, 0:512].rearrange("p (h e) -> p h e", h=H))
    for yc in range(NYC):
        nc.vector.memset(vf_av[yc][:, :, 128:129], 1.0)

    # ragged y=512 row of vf (imag is 0)
    ps512 = ph.tile([1, 512], F32, tag="ph", name=f"psv512_{b}")
    for c in range(NLC):
        nc.tensor.matmul(ps512[0:1, 0:512],
                         fre_sb[:, c, 512:513],
                         v_sb[:, c, :],
                         start=(c == 0), stop=(c == NLC - 1))
    nc.vector.tensor_copy(out=v512[0:1, :, 0:64],
                          in_=ps512[0:1, 0:512].rearrange("p (h e) -> p h e", h=H))
    nc.vector.memset(v512[0:1, :, 64:128], 0.0)
    nc.vector.memset(v512[0:1, :, 128:129], 1.0)
    return vf_av, v512


def _batch(nc, tc, OPS, b, bl, qd, kd, vd, taud, deltad, outd,
           fre_sb, fim_sb, gred, gimd, gre512, gst,
           io, stg, stk, wv, sm, vfp, qkvp, ep, pf, ph, vf_cur):
    vf_av, v512 = vf_cur

    # ---- epilogue scalars -------------------------------------------
    tau_sb = ep.tile([128, 1], F32, tag="tau")
    nc.sync.dma_start(out=tau_sb[:, :], in_=taud[b:b + 1, 0:1].to_broadcast([128, 1]))
    delta_sb = ep.tile([128, NLC], F32, tag="delta")
    nc.sync.dma_start(out=delta_sb[:, :],
                      in_=deltad[b, :].rearrange("(c p) -> p c", p=128))

    # ---- qkv accumulators (written by AV, read by iFFT) -------------
    qkv_all = [qkvp.tile([128, 2, H, 64], BF16, tag="qkv", name=f"qkv{b}_{xc}")
               for xc in range(NXC)]
    qkv512 = qkvp.tile([1, 2, H, 64], BF16, tag="qkv512", bufs=2,
                       name=f"qkv512_{b}")

    for w in range(NWAVE):
        _wave(nc, tc, OPS, b, w, qd, kd, fre_sb, fim_sb,
              io, stg, stk, wv, sm, pf, ph,
              vf_av, v512, qkv_all, qkv512)

    # next batch's independent v-FFT emitted before the iFFT so the PE
    # has work while this batch's softmax/AV tail drains
    vf_next = None
    if b + 1 < bl:
        vf_next = _phase_v(nc, b + 1, vd, fre_sb, fim_sb, io, vfp, pf, ph)

    # ---- iFFT + epilogue --------------------------------------------
    for lc in range(NLC):
        lcs = slice(lc * 128, (lc + 1) * 128)
        gre_lc = gst.tile([128, NXC, 128], BF16, tag="grelc",
                          name=f"grelc{b}_{lc}")
        nc.gpsimd.dma_start(out=gre_lc[:, :, :],
                          in_=gred[0:512, lcs].rearrange("(c p) l -> p c l", p=128))
        gim_lc = gst.tile([128, NXC, 128], BF16, tag="gimlc",
                          name=f"gimlc{b}_{lc}")
        nc.gpsimd.dma_start(out=gim_lc[:, :, :],
                          in_=gimd[0:512, lcs].rearrange("(c p) l -> p c l", p=128))
        ps_o = ph.tile([128, 512], F32, tag="ph", name=f"pso{b}_{lc}")
        for xc in range(NXC):
            nc.tensor.matmul(ps_o[:, 0:512],
                             gre_lc[:, xc, :],
                             qkv_all[xc][:, 0, :, :],
                             start=(xc == 0), stop=False)
            nc.tensor.matmul(ps_o[:, 0:512],
                             gim_lc[:, xc, :],
                             qkv_all[xc][:, 1, :, :],
                             start=False, stop=False)
        nc.tensor.matmul(ps_o[:, 0:512],
                         gre512[0:1, lc * 128:(lc + 1) * 128],
                         qkv512[0:1, 0, :, :],
                         start=False, stop=True)
        out_t = ep.tile([128, 512], F32, tag="outsb", name=f"out{b}_{lc}")
        nc.vector.tensor_scalar(out=out_t[:, :], in0=ps_o[:, 0:512],
                                scalar1=tau_sb[:, 0:1],
                                scalar2=delta_sb[:, lc:lc + 1],
                                op0=OPS.mult, op1=OPS.add)
        nc.sync.dma_start(
            out=outd[b, lc * 128:(lc + 1) * 128, :, :].rearrange("l h e -> l (h e)"),
            in_=out_t[:, :])
    return vf_next


def _wave(nc, tc, OPS, b, w, qd, kd, fre_sb, fim_sb,
          io, stg, stk, wv, sm, pf, ph,
          vf_av, v512, qkv_all, qkv512):
    heads = [4 * w + i for i in range(4)]
    hps = [2 * w, 2 * w + 1]

    # ---- q/k FFT for this wave's head pairs -------------------------
    qstk = {}
    kstk = {}
    kswp = {}
    for hp in hps:
        q_hp = io.tile([128, NLC, 128], BF16, tag="qhp", name=f"qhp{b}_{hp}")
        nc.sync.dma_start(
            out=q_hp[:, :, :],
            in_=qd[b, :, 2 * hp:2 * hp + 2, :].rearrange("(c p) h e -> p c (h e)", p=128))
        k_hp = io.tile([128, NLC, 128], BF16, tag="khp", name=f"khp{b}_{hp}")
        nc.sync.dma_start(
            out=k_hp[:, :, :],
            in_=kd[b, :, 2 * hp:2 * hp + 2, :].rearrange("(c p) h e -> p c (h e)", p=128))

        for t, src, scale in ((0, q_hp, 0.125), (1, k_hp, 1.0)):
            ps_re = pf.tile([128, 520], F32, tag="pf", name=f"psfr{b}_{hp}_{t}")
            ps_im = pf.tile([128, 520], F32, tag="pf", name=f"psfi{b}_{hp}_{t}")
            for c in range(NLC):
                lhsT = src[:, c, :]
                nc.tensor.matmul(ps_re[:, 0:512], lhsT,
                                 fre_sb[:, c, 0:512],
                                 start=(c == 0), stop=(c == NLC - 1))
                nc.tensor.matmul(ps_re[:, 512:514], lhsT,
                                 fre_sb[:, c, 512:514],
                                 start=(c == 0), stop=(c == NLC - 1))
                nc.tensor.matmul(ps_im[:, 0:512], lhsT,
                                 fim_sb[:, c, 0:512],
                                 start=(c == 0), stop=(c == NLC - 1))
                # imag bin 512 is exactly 0 (Fim col 512 is zeros) — computed
                # rather than memset so the tile is f32r-produced throughout.
                nc.tensor.matmul(ps_im[:, 512:514], lhsT,
                                 fim_sb[:, c, 512:514],
                                 start=(c == 0), stop=(c == NLC - 1))
            st_re = stg.tile([128, XP], BF16, tag="stre", name=f"stre{b}_{hp}_{t}")
            nc.vector.tensor_scalar_mul(out=st_re[:, 0:514],
                                        in0=ps_re[:, 0:514], scalar1=scale)
            st_im = stg.tile([128, XP], BF16, tag="stim", name=f"stim{b}_{hp}_{t}")
            nc.vector.tensor_scalar_mul(out=st_im[:, 0:514],
                                        in0=ps_im[:, 0:514], scalar1=scale)
            if t == 1:
                st_imn = stg.tile([128, XP], BF16, tag="stimn",
                                  name=f"stimn{b}_{hp}")
                nc.vector.tensor_scalar_mul(out=st_imn[:, 0:514],
                                            in0=ps_im[:, 0:514], scalar1=-1.0)

            for phi in range(2):
                h = 2 * hp + phi
                rows = slice(64 * phi, 64 * phi + 64)
                if t == 0:
                    dst = stk.tile([128, XP], BF16, tag="qstk", name=f"qstk{b}_{h}")
                    qstk[h] = dst
                    nc.gpsimd.dma_start(out=dst[0:64, :], in_=st_re[rows, :])
                    nc.gpsimd.dma_start(out=dst[64:128, :], in_=st_im[rows, :])
                else:
                    dst = stk.tile([128, XP], BF16, tag="kstk", name=f"kstk{b}_{h}")
                    kstk[h] = dst
                    nc.gpsimd.dma_start(out=dst[0:64, :], in_=st_re[rows, :])
                    nc.gpsimd.dma_start(out=dst[64:128, :], in_=st_im[rows, :])
                    dsw = stk.tile([128, XP], BF16, tag="kswp", name=f"kswp{b}_{h}")
                    kswp[h] = dsw
                    nc.gpsimd.dma_start(out=dsw[0:64, :], in_=st_imn[rows, :])
                    nc.gpsimd.dma_start(out=dsw[64:128, :], in_=st_re[rows, :])

    # ---- QK + squares (both ACT table sets contain `square`) --------
    s_tiles = {}
    rag_s = wv.tile([4, X], F32, tag="rags", bufs=2, name=f"rags{b}_{w}")
    rag_i = wv.tile([4, X], F32, tag="ragi", bufs=2, name=f"ragi{b}_{w}")
    for i, h in enumerate(heads):
        for yc in range(NYC):
            ps_r = pf.tile([128, 520], F32, tag="pf", name=f"psqr{b}_{h}_{yc}")
            ps_i = pf.tile([128, 520], F32, tag="pf", name=f"psqi{b}_{h}_{yc}")
            ycs = slice(yc * 128, (yc + 1) * 128)
            nc.tensor.matmul(ps_r[:, 0:512], kstk[h][:, ycs],
                             qstk[h][:, 0:512], start=True, stop=True)
            nc.tensor.matmul(ps_r[:, 512:514], kstk[h][:, ycs],
                             qstk[h][:, 512:514], start=True, stop=True)
            nc.tensor.matmul(ps_i[:, 0:512], kswp[h][:, ycs],
                             qstk[h][:, 0:512], start=True, stop=True)
            nc.tensor.matmul(ps_i[:, 512:514], kswp[h][:, ycs],
                             qstk[h][:, 512:514], start=True, stop=True)
            s_t = wv.tile([128, X], F32, tag="stile", bufs=17,
                          name=f"st{b}_{h}_{yc}")
            tmp = sm.tile([128, X], F32, tag="sqim", name=f"sqim{b}_{h}_{yc}")
            nc.scalar.square(out=s_t[:, :], in_=ps_r[:, 0:513])
            nc.scalar.square(out=tmp[:, :], in_=ps_i[:, 0:513])
            nc.vector.tensor_add(out=s_t[:, :], in0=s_t[:, :], in1=tmp[:, :])
            s_tiles[(h, yc)] = s_t
        # ragged y = 512 row
        ps_rr = pf.tile([128, 520], F32, tag="pf", name=f"psrr{b}_{h}")
        ps_ri = pf.tile([128, 520], F32, tag="pf", name=f"psri{b}_{h}")
        nc.tensor.matmul(ps_rr[0:1, 0:512], kstk[h][:, 512:513],
                         qstk[h][:, 0:512], start=True, stop=True)
        nc.tensor.matmul(ps_rr[0:1, 512:514], kstk[h][:, 512:513],
                         qstk[h][:, 512:514], start=True, stop=True)
        nc.tensor.matmul(ps_ri[0:1, 0:512], kswp[h][:, 512:513],
                         qstk[h][:, 0:512], start=True, stop=True)
        nc.tensor.matmul(ps_ri[0:1, 512:514], kswp[h][:, 512:513],
                         qstk[h][:, 512:514], start=True, stop=True)
        ra = sm.tile([1, X], F32, tag="sqim", name=f"raga{b}_{h}")
        rb = sm.tile([1, X], F32, tag="sqim", name=f"ragb{b}_{h}")
        nc.vector.tensor_scalar_mul(out=ra[0:1, :], in0=ps_rr[0:1, 0:513],
                                    scalar1=1.0)
        nc.vector.tensor_scalar_mul(out=rb[0:1, :], in0=ps_ri[0:1, 0:513],
                                    scalar1=1.0)
        nc.gpsimd.dma_start(out=rag_s[i:i + 1, :], in_=ra[0:1, :])
        nc.gpsimd.dma_start(out=rag_i[i:i + 1, :], in_=rb[0:1, :])

    # ---- sqrt segment (sqrt table set; `square` is in every set) ----
    nc.scalar.square(out=rag_s[:, :], in_=rag_s[:, :])
    nc.scalar.square(out=rag_i[:, :], in_=rag_i[:, :])
    nc.vector.tensor_add(out=rag_s[:, :], in0=rag_s[:, :], in1=rag_i[:, :])
    for h in heads:
        for yc in range(NYC):
            s_t = s_tiles[(h, yc)]
            nc.scalar.sqrt(out=s_t[:, :], in_=s_t[:, :])
    nc.scalar.sqrt(out=rag_s[:, :], in_=rag_s[:, :])

    # ---- exp segment (exp table set) --------------------------------
    # exp is written bf16 IN PLACE over the fp32 magnitude tile (bitcast
    # view): the 2-byte writes trail the 4-byte reads in stream order.
    exp_tiles = {}
    for h in heads:
        for yc in range(NYC):
            s_t = s_tiles[(h, yc)]
            e_view = s_t.bitcast(BF16)
            nc.scalar.activation(out=e_view[:, 0:X], in_=s_t[:, :],
                                 func=AF.Exp)
            exp_tiles[(h, yc)] = e_view
    rag_p = wv.tile([4, X], BF16, tag="ragp", bufs=2, name=f"ragp{b}_{w}")
    nc.scalar.activation(out=rag_p[:, :], in_=rag_s[:, :], func=AF.Exp)
    exp_rag = {}
    for i, h in enumerate(heads):
        er = wv.tile([1, X], BF16, tag="exprag", bufs=6, name=f"er{b}_{h}")
        exp_rag[h] = er
        nc.gpsimd.dma_start(out=er[0:1, :], in_=rag_p[i:i + 1, :])

    # ---- AV + colsum normalization ----------------------------------
    for h in heads:
        for xc in range(NXC):
            xcs = slice(xc * 128, (xc + 1) * 128)
            ps_av = ph.tile([128, 512], F32, tag="ph", name=f"psav{b}_{h}_{xc}")
            for yc in range(NYC):
                nc.tensor.matmul(ps_av[:, 0:129], exp_tiles[(h, yc)][:, xcs],
                                 vf_av[yc][:, h, 0:129],
                                 start=(yc == 0), stop=False)
            nc.tensor.matmul(ps_av[:, 0:129], exp_rag[h][0:1, xcs],
                             v512[0:1, h, 0:129], start=False, stop=True)
            rc = sm.tile([128, 1], F32, tag="rc", bufs=4, name=f"rc{b}_{h}_{xc}")
            nc.vector.reciprocal(out=rc[:, :], in_=ps_av[:, 128:129])
            nc.vector.tensor_scalar_mul(
                out=qkv_all[xc][:, :, h, :],
                in0=ps_av[:, 0:128].rearrange("p (t e) -> p t e", t=2),
                scalar1=rc[:, 0:1])
        # ragged x = 512 row
        ps_a1 = ph.tile([128, 512], F32, tag="ph", name=f"psa1{b}_{h}")
        for yc in range(NYC):
            nc.tensor.matmul(ps_a1[0:1, 0:129], exp_tiles[(h, yc)][:, 512:513],
                             vf_av[yc][:, h, 0:129],
                             start=(yc == 0), stop=False)
        nc.tensor.matmul(ps_a1[0:1, 0:129], exp_rag[h][0:1, 512:513],
                         v512[0:1, h, 0:129], start=False, stop=True)
        rc1 = sm.tile([1, 1], F32, tag="rc1", bufs=2, name=f"rc1{b}_{h}")
        nc.vector.reciprocal(out=rc1[0:1, :], in_=ps_a1[0:1, 128:129])
        nc.vector.tensor_scalar_mul(
            out=qkv512[0:1, :, h, :],
            in0=ps_a1[0:1, 0:128].rearrange("p (t e) -> p t e", t=2),
            scalar1=rc1[0:1, 0:1])


AF = mybir.ActivationFunctionType

_BUILT = None
_CONSTS = None


def _get_built():
    global _BUILT, _CONSTS
    if _BUILT is None:
        _BUILT = build_module()
        _CONSTS = _consts()
    return _BUILT, _CONSTS


def kernel(q, k, v, mask, tau, delta):
    global LAST_RESULT
    nc, (fre, fim, gre, gim) = _get_built()
    q = np.ascontiguousarray(np.asarray(q, dtype=np.float32)).astype(ml_dtypes.bfloat16)
    k = np.ascontiguousarray(np.asarray(k, dtype=np.float32)).astype(ml_dtypes.bfloat16)
    v = np.ascontiguousarray(np.asarray(v, dtype=np.float32)).astype(ml_dtypes.bfloat16)
    tau = np.ascontiguousarray(np.asarray(tau, dtype=np.float32))
    delta = np.ascontiguousarray(np.asarray(delta, dtype=np.float32))

    in_maps = []
    for i in range(NCORES):
        sl = slice(i * BL, (i + 1) * BL)
        in_maps.append({
            "qd": np.ascontiguousarray(q[sl]),
            "kd": np.ascontiguousarray(k[sl]),
            "vd": np.ascontiguousarray(v[sl]),
            "taud": np.ascontiguousarray(tau[sl]),
            "deltad": np.ascontiguousarray(delta[sl]),
            "fred": fre, "fimd": fim, "gred": gre, "gimd": gim,
        })
    res = run_bass_kernel_spmd(nc, in_maps, core_ids=list(range(NCORES)))
    LAST_RESULT = res
    out = np.concatenate([res.results[i]["outd"] for i in range(NCORES)], axis=0)
    return out.astype(np.float32)

